# revision 1
# baseline (speedup 1.0000x reference)
"""MoE (BruteForceMoELinear) Trainium2 kernel.

Strategy: expert-parallel across 8 NeuronCores. The host (inside
`kernel()`) dispatches token rows by `gate_idx` (stable sort), pads each
expert's token batch to a common capacity C, and hands core e:

  xt  : (128, 4, C)    = x_e^T   laid out [d_inner, d_outer, token]
  w1t : (128, 4, 2048) = W1_e^T  laid out [d_inner, d_outer, f]
  w2t : (128, 16, 512) = W2_e^T  laid out [f_inner, f_outer, d_out]
  sc  : (128, C)       = per-token gate score, replicated over partitions

Each core computes  y_e^T = (W2_e @ relu(W1_e @ x_e^T)) * score  with
float32r matmuls (full-rate fp32 PE path), ReLU fused into the PSUM
eviction on the scalar engine and the gate-score multiply fused into the
second GEMM's PSUM eviction on the vector engine.  The host scatters the
per-expert outputs back to token order and sums the top-k (=2) slots.
"""

import numpy as np

NUM_EXPERT = 8
N_CORES = 8
P = 128

_CACHE = {}


def _build(TN, NCH, KO, FO, repeat=1):
    """Compile the per-core program for capacity C = TN*NCH tokens.

    KO = d_model/128, FO = d_ff/128.  `repeat` re-emits the compute body
    (used only for timing calibration in the dev harness).
    """
    key = (TN, NCH, KO, FO, repeat)
    if key in _CACHE:
        return _CACHE[key]

    import concourse.mybir as mybir
    import concourse.tile as tile
    from concourse import bacc

    f32 = mybir.dt.float32
    f32r = mybir.dt.float32r
    C = TN * NCH
    D_MODEL = KO * P
    D_FF = FO * P

    nc = bacc.Bacc("TRN2", target_bir_lowering=False, debug=False,
                   num_devices=N_CORES)

    xt = nc.dram_tensor("xt", (P, KO, C), f32r, kind="ExternalInput")
    w1t = nc.dram_tensor("w1t", (P, KO, D_FF), f32r, kind="ExternalInput")
    w2t = nc.dram_tensor("w2t", (P, FO, D_MODEL), f32r, kind="ExternalInput")
    sc = nc.dram_tensor("sc", (P, C), f32, kind="ExternalInput")
    yt = nc.dram_tensor("yt", (P, KO, C), f32, kind="ExternalOutput")

    # Holding every chunk's h in SBUF only fits for NCH <= 2; for heavily
    # skewed expert distributions (NCH >= 3) process chunk-major with a
    # rotating 2-buffer h pool instead.
    # NOTE: pools reserve bufs slots PER TAG; the NCH<=2 path uses one
    # persistent tile per chunk tag, so 1 slot per tag suffices (bufs=NCH
    # would double-reserve and overflow SBUF around TN>=400, NCH=2).
    NHB = 1 if NCH <= 2 else 2
    NXB = 1 if NCH <= 2 else 3
    with tile.TileContext(nc) as tc:
        with tc.tile_pool(name="wpool", bufs=1) as wpool, \
             tc.tile_pool(name="xpool", bufs=NXB) as xpool, \
             tc.tile_pool(name="hpool", bufs=NHB) as hpool, \
             tc.tile_pool(name="ypool", bufs=4) as ypool, \
             tc.tile_pool(name="cpool", bufs=1) as cpool, \
             tc.tile_pool(name="ps1", bufs=6, space="PSUM") as ps1, \
             tc.tile_pool(name="ps2", bufs=2, space="PSUM") as ps2:

            bias0 = cpool.tile([P, 1], f32)
            nc.any.memset(bias0[:], 0.0)

            # PE warm-up: dummy matmuls on memset data keep the PE busy
            # through the DMA-priming window so the HAM clock gate is at
            # full rate when the first real matmul issues.
            warm = cpool.tile([P, 64], f32)
            nc.any.memset(warm[:], 0.5)
            wps = ps1.tile([P, 64], f32, name="warm", tag="p1")
            for _i in range(20):
                nc.tensor.matmul(wps[:64, :], warm[:], warm[:],
                                 start=True, stop=True)

            # DMAs execute in emission order on the DMA stream, which is
            # the pacing resource at kernel start.  Emit strictly in
            # consumption order: x(ch0) -> W1 -> x(ch1..) -> W2/sc.
            w1sb = wpool.tile([P, KO, D_FF], f32r)
            w2sb = wpool.tile([P, FO, D_MODEL], f32r)
            scsb = cpool.tile([P, C], f32)
            if NCH <= 2:
                xsbs = [xpool.tile([P, KO, TN], f32r, tag=f"x{ch}",
                                   name=f"xsb{ch}") for ch in range(NCH)]
            else:
                xsbs = None  # allocated per chunk in the fallback loop

            # DMA emission order == consumption order: x/W1 for the first
            # f-block pairwise (fine-grained so the first fo-group starts
            # after ~3 small DMAs), later chunks' x, the rest of W1, then
            # W2 d-blocks and the gate scores.
            FB = 512
            NFB = D_FF // FB
            FPB = FB // P  # fo-groups per W1 f-block
            if NCH <= 2:
                nc.sync.dma_start(w1sb[:, 0:2, 0:FB],
                                  w1t.ap()[:, 0:2, 0:FB])
                nc.sync.dma_start(xsbs[0][:], xt.ap()[:, :, 0:TN])
                nc.sync.dma_start(w1sb[:, 2:KO, 0:FB],
                                  w1t.ap()[:, 2:KO, 0:FB])
                for ch in range(1, NCH):
                    nc.sync.dma_start(xsbs[ch][:],
                                      xt.ap()[:, :, ch * TN:(ch + 1) * TN])
            else:
                nc.sync.dma_start(w1sb[:, :, 0:FB], w1t.ap()[:, :, 0:FB])
            HB = FB // 4
            for hb in range(4, 4 * NFB):
                nc.sync.dma_start(
                    w1sb[:, :, hb * HB:(hb + 1) * HB],
                    w1t.ap()[:, :, hb * HB:(hb + 1) * HB])
            nc.sync.dma_start(w2sb[:, :, 0:P], w2t.ap()[:, :, 0:P])
            nc.sync.dma_start(scsb[:], sc.ap())
            for db in range(1, KO):
                nc.sync.dma_start(w2sb[:, :, db * P:(db + 1) * P],
                                  w2t.ap()[:, :, db * P:(db + 1) * P])

            def gemm1(hsb, xsb, fo):
                p1 = ps1.tile([P, TN], f32, name="p1", tag="p1")
                for ko in range(KO):
                    nc.tensor.matmul(
                        p1[:],
                        w1sb[:, ko, fo * P:(fo + 1) * P],
                        xsb[:, ko, :],
                        start=(ko == 0), stop=(ko == KO - 1))
                nc.scalar.activation(
                    hsb[:, fo, :], p1[:],
                    mybir.ActivationFunctionType.Relu, bias=bias0[:])

            def gemm2(hsb, do, tsl):
                p2 = ps2.tile([P, TN], f32, name="p2", tag="p2")
                for fo in range(FO):
                    nc.tensor.matmul(
                        p2[:],
                        w2sb[:, fo, do * P:(do + 1) * P],
                        hsb[:, fo, :],
                        start=(fo == 0), stop=(fo == FO - 1))
                ysb = ypool.tile([P, TN], f32, tag="y", name="ysb")
                nc.vector.tensor_mul(ysb[:], p2[:], scsb[:, tsl])
                nc.sync.dma_start(yt.ap()[:, do, tsl], ysb[:])

            for _ in range(repeat):
                if NCH <= 2:
                    hsbs = [hpool.tile([P, FO, TN], f32r, tag=f"h{ch}",
                                       name=f"hsb{ch}") for ch in range(NCH)]
                    # phase 1: h = relu(W1 @ x^T); f-block-major so every
                    # W1 block feeds all chunks' matmuls before the next
                    # block is needed (keeps PE ahead of the DMA stream).
                    for fb in range(NFB):
                        for ch in range(NCH):
                            for fo in range(fb * FPB, (fb + 1) * FPB):
                                gemm1(hsbs[ch], xsbs[ch], fo)
                    # phase 2: y^T = (W2 @ h) * score; d-block-major,
                    # streamed out per (db, chunk).
                    for do in range(KO):
                        for ch in range(NCH):
                            gemm2(hsbs[ch], do,
                                  slice(ch * TN, (ch + 1) * TN))
                else:
                    # chunk-major fallback (bounded SBUF for large NCH)
                    for ch in range(NCH):
                        xsb = xpool.tile([P, KO, TN], f32r, tag="x",
                                         name="xsb")
                        nc.sync.dma_start(
                            xsb[:], xt.ap()[:, :, ch * TN:(ch + 1) * TN])
                        hsb = hpool.tile([P, FO, TN], f32r, tag="h",
                                         name="hsb")
                        for fo in range(FO):
                            gemm1(hsb, xsb, fo)
                        for do in range(KO):
                            gemm2(hsb, do, slice(ch * TN, (ch + 1) * TN))

    nc.compile()
    _CACHE[key] = nc
    return nc


def _capacity(max_count):
    """Chunking: NCH chunks of TN tokens; TN in [256, 512] keeps the
    float32r matmul at full rate and within one PSUM bank."""
    maxc = max(int(max_count), 1)
    nch = -(-maxc // 512)
    tn = -(-maxc // (nch * 8)) * 8
    tn = max(tn, 256)
    return tn, nch


_last = {}


def kernel(inp, gate_idx, gate_score, w_htoh4, w_h4toh):
    inp = np.ascontiguousarray(np.asarray(inp, dtype=np.float32))
    gate_idx = np.asarray(gate_idx)
    gate_score = np.asarray(gate_score, dtype=np.float32)
    w_htoh4 = np.asarray(w_htoh4, dtype=np.float32)
    w_h4toh = np.asarray(w_h4toh, dtype=np.float32)

    B, d_model = inp.shape
    n_expert, d_ff, _ = w_htoh4.shape
    assert n_expert == NUM_EXPERT
    KO = d_model // P
    FO = d_ff // P

    gi = gate_idx.astype(np.int64)
    order = np.argsort(gi, kind="stable")
    counts = np.bincount(gi, minlength=NUM_EXPERT)
    idx_split = np.split(order, np.cumsum(counts)[:-1])

    TN, NCH = _capacity(counts.max())
    C = TN * NCH

    # flat per-row gate scores: row 2n+k of inp gets gate_score[n, 0, k]
    scores_flat = gate_score.reshape(-1)

    nc = _build(TN, NCH, KO, FO)

    in_maps = []
    for e in range(NUM_EXPERT):
        idx = idx_split[e]
        cnt = len(idx)
        xT = np.zeros((d_model, C), dtype=np.float32)
        if cnt:
            xT[:, :cnt] = inp[idx].T
        xt_h = np.ascontiguousarray(
            xT.reshape(KO, P, C).transpose(1, 0, 2))
        w1_h = np.ascontiguousarray(
            w_htoh4[e].T.reshape(KO, P, d_ff).transpose(1, 0, 2))
        w2_h = np.ascontiguousarray(
            w_h4toh[e].T.reshape(FO, P, d_model).transpose(1, 0, 2))
        sc_vec = np.zeros((C,), dtype=np.float32)
        if cnt:
            sc_vec[:cnt] = scores_flat[idx]
        sc_h = np.ascontiguousarray(np.broadcast_to(sc_vec, (P, C)))
        in_maps.append({"xt": xt_h, "w1t": w1_h, "w2t": w2_h, "sc": sc_h})

    from concourse import bass_utils
    res = bass_utils.run_bass_kernel_spmd(nc, in_maps,
                                          core_ids=list(range(N_CORES)))

    _last.update(nc=nc, in_maps=in_maps, res=res, TN=TN, NCH=NCH,
                 KO=KO, FO=FO)

    y_full = np.empty((B, d_model), dtype=np.float32)
    for e in range(NUM_EXPERT):
        idx = idx_split[e]
        if len(idx) == 0:
            continue
        yt_h = res.results[e]["yt"]  # (P, KO, C)
        yT = yt_h.transpose(1, 0, 2).reshape(d_model, C)
        y_full[idx] = yT[:, :len(idx)].T

    out = y_full[0::2] + y_full[1::2]
    return np.ascontiguousarray(out, dtype=np.float32)



# revision 2
# speedup vs baseline: 1.0303x; 1.0303x over previous
"""MoE (BruteForceMoELinear) Trainium2 kernel — expert-parallel, bf16.

Strategy: one expert per NeuronCore (8 experts / 8 cores).  The host
(inside `kernel()`) dispatches token rows by `gate_idx`, folds the gate
score into the tokens (s >= 0, so relu(W1 (s x)) = s relu(W1 x) and the
whole per-token scale commutes through both GEMMs), pads each expert's
batch to a common capacity C, converts everything to bf16 and hands
core e:

  pk  : (128, KO, c0+256)  = [x_e^T chunk0 | W1_e^T f-cols 0:256]
  w1r : (128, KO, d_ff-256)= W1_e^T f-cols 256:d_ff
  xt1 : (128, KO, C-c0)    = x_e^T remaining chunks
  w2t : (128, KO, FO, 128) = W2_e^T blocked [f_in, d_out_blk, f_blk, d_in]

Each core computes  y_e^T = W2_e @ relu(W1_e @ x_e^T)  with bf16
matmuls (full-rate PE) accumulating in f32 PSUM; ReLU is fused into the
PSUM eviction (scalar engine, bf16 out) and the final y is evicted f32
and DMA'd out per (d-block, chunk).  The pk pack rides the Pool-engine
SWDGE path so its descriptor generation overlaps the HWDGE pipeline of
the main weight stream, putting real data on the PE ~3.3 us after
launch; dummy warm-up matmuls keep the PE p-state ramp burned by then.
The host scatters per-expert outputs back to token order and sums the
top-k (=2) slots.
"""

import numpy as np

NUM_EXPERT = 8
N_CORES = 8
P = 128
FPACK = 256          # w1 f-columns packed with x chunk0
NWARM = 52           # PE p-state warm-up matmuls (64 rows each)

_CACHE = {}


def _chunking(maxc):
    """Token capacity C (multiple of 8) and chunk sizes (<=504 each,
    first chunk 256 when possible so the packed DMA stays small)."""
    c = max(-(-int(maxc) // 8) * 8, 16)
    if c <= 504:
        return [c] if c <= 256 else [256, c - 256]
    chunks = [256]
    rem = c - 256
    n = -(-rem // 504)
    base = -(-rem // (n * 8)) * 8
    while rem > 0:
        t = min(base, rem)
        chunks.append(t)
        rem -= t
    return chunks


def _build(chunks, KO, FO, repeat=1):
    key = (tuple(chunks), KO, FO, repeat)
    if key in _CACHE:
        return _CACHE[key]

    import concourse.mybir as mybir
    import concourse.tile as tile
    from concourse import bacc

    f32 = mybir.dt.float32
    bf16 = mybir.dt.bfloat16
    C = sum(chunks)
    c0 = chunks[0]
    D_FF = FO * P
    W1R = D_FF - FPACK
    NCH = len(chunks)
    offs = np.cumsum([0] + list(chunks))

    nc = bacc.Bacc("TRN2", target_bir_lowering=False, debug=False,
                   num_devices=N_CORES)

    pk = nc.dram_tensor("pk", (P, KO, c0 + FPACK), bf16, kind="ExternalInput")
    w1r = nc.dram_tensor("w1r", (P, KO, W1R), bf16, kind="ExternalInput")
    if NCH > 1:
        xt1 = nc.dram_tensor("xt1", (P, KO, C - c0), bf16,
                             kind="ExternalInput")
    w2t = nc.dram_tensor("w2t", (P, KO, FO, P), bf16, kind="ExternalInput")
    yt = nc.dram_tensor("yt", (P, KO, C), f32, kind="ExternalOutput")

    NPB = 2 if NCH <= 2 else 1  # PSUM bufs per tag (8 banks total)
    with tile.TileContext(nc) as tc:
        with tc.tile_pool(name="wpool", bufs=1) as wpool, \
             tc.tile_pool(name="ypool", bufs=2) as ypool, \
             tc.tile_pool(name="ps1", bufs=NPB, space="PSUM") as ps1, \
             tc.tile_pool(name="ps2", bufs=NPB, space="PSUM") as ps2:

            bias0 = wpool.tile([P, 1], f32)
            nc.vector.memset(bias0[:], 0.0)
            warm = wpool.tile([P, 64], bf16)
            nc.vector.memset(warm[:], 0.5)

            # Pool-engine (SWDGE) DMA for the x|w1-head pack: descriptor
            # generation runs on the Pool engine, in parallel with the
            # HWDGE descriptor generation of the SP-queue stream below.
            pkt = wpool.tile([P, KO, c0 + FPACK], bf16)
            nc.gpsimd.dma_start(pkt[:], pk.ap())

            # PE p-state warm-up: burns the ramp (full clock needs ~3 us
            # from first PE activity) while the first DMAs land.
            wps = ps1.tile([P, c0], f32, name="warm", tag="p1c0")
            for _ in range(NWARM):
                nc.tensor.matmul(wps[:64, :64], warm[:], warm[:],
                                 start=True, stop=True)

            # SP-queue DMA stream, in consumption order.  w1 head
            # (f 0:256) arrives via pk; stream the rest in 256-col
            # pieces, with the remaining x chunks early.
            w1sb = wpool.tile([P, KO, W1R], bf16)
            w2sb = wpool.tile([P, KO, FO, P], bf16)
            FB = 256
            nc.sync.dma_start(w1sb[:, :, 0:FB], w1r.ap()[:, :, 0:FB])
            nc.sync.dma_start(w1sb[:, :, FB:2 * FB],
                              w1r.ap()[:, :, FB:2 * FB])
            if NCH > 1:
                xsb1 = wpool.tile([P, KO, C - c0], bf16)
                nc.sync.dma_start(xsb1[:], xt1.ap())
            for fb in range(2, W1R // FB):
                nc.sync.dma_start(w1sb[:, :, fb * FB:(fb + 1) * FB],
                                  w1r.ap()[:, :, fb * FB:(fb + 1) * FB])
            for do in range(KO):
                nc.sync.dma_start(w2sb[:, do], w2t.ap()[:, do])

            def w1_lhsT(ko, fo):
                if fo * P < FPACK:
                    return pkt[:, ko, c0 + fo * P:c0 + (fo + 1) * P]
                return w1sb[:, ko, fo * P - FPACK:(fo + 1) * P - FPACK]

            def x_rhs(ch, ko):
                if ch == 0:
                    return pkt[:, ko, 0:c0]
                return xsb1[:, ko, offs[ch] - c0:offs[ch + 1] - c0]

            hs = [wpool.tile([P, FO, chunks[ch]], bf16, name=f"h{ch}")
                  for ch in range(NCH)]

            def gemm1(ch, fo):
                p1 = ps1.tile([P, chunks[ch]], f32, name="p1",
                              tag=f"p1c{ch}")
                for ko in range(KO):
                    nc.tensor.matmul(p1[:], w1_lhsT(ko, fo), x_rhs(ch, ko),
                                     start=(ko == 0), stop=(ko == KO - 1))
                nc.scalar.activation(hs[ch][:, fo, :], p1[:],
                                     mybir.ActivationFunctionType.Relu,
                                     bias=bias0[:])

            def gemm2(ch, do):
                p2 = ps2.tile([P, chunks[ch]], f32, name="p2",
                              tag=f"p2c{ch}")
                for fo in range(FO):
                    nc.tensor.matmul(p2[:], w2sb[:, do, fo, :],
                                     hs[ch][:, fo, :],
                                     start=(fo == 0), stop=(fo == FO - 1))
                ysb = ypool.tile([P, chunks[ch]], f32, tag=f"y{ch}",
                                 name="ysb")
                nc.scalar.copy(ysb[:], p2[:])
                nc.sync.dma_start(yt.ap()[:, do, offs[ch]:offs[ch + 1]],
                                  ysb[:])

            for _ in range(repeat):
                # Phase 1: h = relu(W1 x).  Head f-blocks chunk-major so
                # chunk 1+'s x DMA can land; tail f-blocks fo-major.
                HEAD = 6 if NCH > 1 else FO
                for ch in range(NCH):
                    for fo in range(min(HEAD, FO)):
                        gemm1(ch, fo)
                for fo in range(HEAD, FO):
                    for ch in range(NCH):
                        gemm1(ch, fo)
                # Phase 2: y = W2 h, d-block-major.  Last do ends on
                # chunk 0 (the small chunk) to shorten the exit chain.
                for do in range(KO):
                    order = range(NCH) if do < KO - 1 else \
                        list(range(1, NCH)) + [0]
                    for ch in order:
                        gemm2(ch, do)

    nc.compile()
    _CACHE[key] = nc
    return nc


_last = {}


def kernel(inp, gate_idx, gate_score, w_htoh4, w_h4toh):
    import ml_dtypes

    bf16 = ml_dtypes.bfloat16
    inp = np.asarray(inp, dtype=np.float32)
    gate_idx = np.asarray(gate_idx)
    gate_score = np.asarray(gate_score, dtype=np.float32)
    w_htoh4 = np.asarray(w_htoh4, dtype=np.float32)
    w_h4toh = np.asarray(w_h4toh, dtype=np.float32)

    B, d_model = inp.shape
    n_expert, d_ff, _ = w_htoh4.shape
    assert n_expert == NUM_EXPERT
    KO = d_model // P
    FO = d_ff // P

    gi = gate_idx.astype(np.int64)
    order = np.argsort(gi, kind="stable")
    counts = np.bincount(gi, minlength=NUM_EXPERT)
    idx_split = np.split(order, np.cumsum(counts)[:-1])

    chunks = _chunking(counts.max())
    C = sum(chunks)
    c0 = chunks[0]

    # fold per-row gate score into x (scores >= 0 commute with relu)
    scores_flat = gate_score.reshape(-1)
    xs = inp * scores_flat[:, None]

    nc = _build(chunks, KO, FO)

    in_maps = []
    for e in range(NUM_EXPERT):
        idx = idx_split[e]
        cnt = len(idx)
        xT = np.zeros((d_model, C), dtype=np.float32)
        if cnt:
            xT[:, :cnt] = xs[idx].T
        xt_h = xT.reshape(KO, P, C).transpose(1, 0, 2).astype(bf16)
        w1_h = np.ascontiguousarray(
            w_htoh4[e].T.reshape(KO, P, d_ff).transpose(1, 0, 2)).astype(bf16)
        # W2^T blocked: w2t[p, do, fo, d] = W2[do*128+d, fo*128+p]
        w2_h = np.ascontiguousarray(
            w_h4toh[e].T.reshape(FO, P, KO, P).transpose(1, 2, 0, 3)
        ).astype(bf16)
        pk_h = np.ascontiguousarray(
            np.concatenate([xt_h[:, :, :c0], w1_h[:, :, :FPACK]], axis=2))
        m = {"pk": pk_h,
             "w1r": np.ascontiguousarray(w1_h[:, :, FPACK:]),
             "w2t": w2_h}
        if len(chunks) > 1:
            m["xt1"] = np.ascontiguousarray(xt_h[:, :, c0:])
        in_maps.append(m)

    from concourse import bass_utils
    res = bass_utils.run_bass_kernel_spmd(nc, in_maps,
                                          core_ids=list(range(N_CORES)))

    _last.update(nc=nc, in_maps=in_maps, res=res, chunks=chunks,
                 KO=KO, FO=FO)

    y_full = np.empty((B, d_model), dtype=np.float32)
    for e in range(NUM_EXPERT):
        idx = idx_split[e]
        if len(idx) == 0:
            continue
        yt_h = np.asarray(res.results[e]["yt"], dtype=np.float32)
        yT = yt_h.transpose(1, 0, 2).reshape(d_model, C)
        y_full[idx] = yT[:, :len(idx)].T

    out = y_full[0::2] + y_full[1::2]
    return np.ascontiguousarray(out, dtype=np.float32)


# revision 3
# speedup vs baseline: 1.0318x; 1.0014x over previous
"""MoE (BruteForceMoELinear) Trainium2 kernel — expert-parallel, bf16.

Strategy: one expert per NeuronCore (8 experts / 8 cores).  The host
(inside `kernel()`) dispatches token rows by `gate_idx`, folds the gate
score into the tokens (s >= 0, so relu(W1 (s x)) = s relu(W1 x) and the
whole per-token scale commutes through both GEMMs), pads each expert's
batch to a common capacity C, converts everything to bf16 and hands
core e:

  xt  : (128, KO, C)       = x_e^T (pre-scaled by gate score)
  w1t : (128, KO, d_ff)    = W1_e^T
  w2t : (128, KO, FO, 128) = W2_e^T blocked [f_in, d_blk, f_blk, d_in]

Each core computes  y_e^T = W2_e @ relu(W1_e @ x_e^T)  with bf16
matmuls (full-rate PE) accumulating in f32 PSUM; ReLU is fused into the
PSUM eviction (scalar engine, bf16 out).  The x chunk rides the SP
HWDGE queue while the W1 head block rides the Pool-engine SWDGE queue,
so the two descriptor-generation pipelines overlap and real data hits
the PE ~4.3 us after launch (DMA-complete semaphores cost +900 ns
each); dummy warm-up matmuls keep the PE p-state ramp burned with <100
ns of PE idle before the first real matmul (idle gaps > ~1 us reset the
ramp).  The host scatters per-expert outputs back to token order and
sums the top-k (=2) slots.
"""

import numpy as np

NUM_EXPERT = 8
N_CORES = 8
P = 128
NWARM = 60           # PE p-state warm-up matmuls (64 rows each)
TAILSPLIT = 64       # tokens in the final PSUM group (shortens exit)

_CACHE = {}


def _chunking(maxc):
    """Token capacity C (multiple of 8) and chunk sizes (<=504 each,
    first chunk 256 when possible: 256 bf16 tokens = 512 B contiguous
    DMA runs, the smallest transfer at full DMA efficiency)."""
    c = max(-(-int(maxc) // 8) * 8, 16)
    if c <= 504:
        return [c] if c <= 256 else [256, c - 256]
    chunks = [256]
    rem = c - 256
    n = -(-rem // 504)
    base = -(-rem // (n * 8)) * 8
    while rem > 0:
        t = min(base, rem)
        chunks.append(t)
        rem -= t
    return chunks


def _build(chunks, KO, FO, repeat=1):
    key = (tuple(chunks), KO, FO, repeat)
    if key in _CACHE:
        return _CACHE[key]

    import concourse.mybir as mybir
    import concourse.tile as tile
    from concourse import bacc

    f32 = mybir.dt.float32
    bf16 = mybir.dt.bfloat16
    C = sum(chunks)
    c0 = chunks[0]
    D_FF = FO * P
    NCH = len(chunks)
    offs = np.cumsum([0] + list(chunks))

    nc = bacc.Bacc("TRN2", target_bir_lowering=False, debug=False,
                   num_devices=N_CORES)

    xt = nc.dram_tensor("xt", (P, KO, C), bf16, kind="ExternalInput")
    w1t = nc.dram_tensor("w1t", (P, KO, D_FF), bf16, kind="ExternalInput")
    w2t = nc.dram_tensor("w2t", (P, KO, FO, P), bf16, kind="ExternalInput")
    yt = nc.dram_tensor("yt", (P, KO, C), f32, kind="ExternalOutput")

    NPB = 2 if NCH <= 2 else 1  # PSUM bufs per tag (8 banks total)
    with tile.TileContext(nc) as tc:
        with tc.tile_pool(name="wpool", bufs=1) as wpool, \
             tc.tile_pool(name="ypool", bufs=2) as ypool, \
             tc.tile_pool(name="ps1", bufs=NPB, space="PSUM") as ps1, \
             tc.tile_pool(name="ps2", bufs=NPB, space="PSUM") as ps2:

            bias0 = wpool.tile([P, 1], f32)
            nc.vector.memset(bias0[:], 0.0)
            warm = wpool.tile([P, 64], bf16)
            nc.vector.memset(warm[:], 0.5)

            xsb = wpool.tile([P, KO, C], bf16)
            w1sb = wpool.tile([P, KO, D_FF], bf16)
            w2sb = wpool.tile([P, KO, FO, P], bf16)

            # x chunk0 on the SP/HWDGE queue, W1 head (f 0:256) on the
            # Pool/SWDGE queue: the two descriptor-generation pipelines
            # run in parallel and the transfers queue back-to-back.
            FB = 256
            nc.sync.dma_start(xsb[:, :, 0:c0], xt.ap()[:, :, 0:c0])
            nc.gpsimd.dma_start(w1sb[:, :, 0:FB], w1t.ap()[:, :, 0:FB])

            # PE p-state warm-up: burns the ramp (full clock needs ~3 us
            # from first PE activity) while the first DMAs land.
            wps = ps1.tile([P, c0], f32, name="warm", tag="p1c0")
            for _ in range(NWARM):
                nc.tensor.matmul(wps[:64, :64], warm[:], warm[:],
                                 start=True, stop=True)

            # Rest of the SP stream, in consumption order.
            nc.sync.dma_start(w1sb[:, :, FB:2 * FB],
                              w1t.ap()[:, :, FB:2 * FB])
            nc.sync.dma_start(w1sb[:, :, 2 * FB:3 * FB],
                              w1t.ap()[:, :, 2 * FB:3 * FB])
            if NCH > 1:
                nc.sync.dma_start(xsb[:, :, c0:], xt.ap()[:, :, c0:])
            for fb in range(3, D_FF // FB):
                nc.sync.dma_start(w1sb[:, :, fb * FB:(fb + 1) * FB],
                                  w1t.ap()[:, :, fb * FB:(fb + 1) * FB])
            for do in range(KO):
                nc.sync.dma_start(w2sb[:, do], w2t.ap()[:, do])

            hs = [wpool.tile([P, FO, chunks[ch]], bf16, name=f"h{ch}")
                  for ch in range(NCH)]

            def gemm1(ch, fo):
                p1 = ps1.tile([P, chunks[ch]], f32, name="p1",
                              tag=f"p1c{ch}")
                for ko in range(KO):
                    nc.tensor.matmul(p1[:], w1sb[:, ko, fo * P:(fo + 1) * P],
                                     xsb[:, ko, offs[ch]:offs[ch + 1]],
                                     start=(ko == 0), stop=(ko == KO - 1))
                nc.scalar.activation(hs[ch][:, fo, :], p1[:],
                                     mybir.ActivationFunctionType.Relu,
                                     bias=bias0[:])

            def gemm2(ch, do, lo=0, hi=None):
                hi = chunks[ch] if hi is None else hi
                n = hi - lo
                p2 = ps2.tile([P, chunks[ch]], f32, name="p2",
                              tag=f"p2c{ch}")
                for fo in range(FO):
                    nc.tensor.matmul(p2[:, 0:n], w2sb[:, do, fo, :],
                                     hs[ch][:, fo, lo:hi],
                                     start=(fo == 0), stop=(fo == FO - 1))
                ysb = ypool.tile([P, chunks[ch]], f32, tag=f"y{ch}",
                                 name="ysb")
                nc.scalar.copy(ysb[:, 0:n], p2[:, 0:n])
                nc.sync.dma_start(
                    yt.ap()[:, do, offs[ch] + lo:offs[ch] + hi],
                    ysb[:, 0:n])

            for _ in range(repeat):
                # Phase 1: h = relu(W1 x).  Head f-blocks chunk-major so
                # chunk 1+'s x DMA can land; tail f-blocks fo-major.
                HEAD = 6 if NCH > 1 else FO
                for ch in range(NCH):
                    for fo in range(min(HEAD, FO)):
                        gemm1(ch, fo)
                for fo in range(HEAD, FO):
                    for ch in range(NCH):
                        gemm1(ch, fo)
                # Phase 2: y = W2 h, d-block-major.  The last do block
                # ends on chunk 0, split so the final PSUM group (and
                # its eviction + DMA) covers only TAILSPLIT tokens.
                for do in range(KO):
                    if do < KO - 1:
                        for ch in range(NCH):
                            gemm2(ch, do)
                    else:
                        for ch in range(1, NCH):
                            gemm2(ch, do)
                        if c0 > TAILSPLIT:
                            gemm2(0, do, 0, c0 - TAILSPLIT)
                            gemm2(0, do, c0 - TAILSPLIT, c0)
                        else:
                            gemm2(0, do)

    nc.compile()
    _CACHE[key] = nc
    return nc


_last = {}


def kernel(inp, gate_idx, gate_score, w_htoh4, w_h4toh):
    import ml_dtypes

    bf16 = ml_dtypes.bfloat16
    inp = np.asarray(inp, dtype=np.float32)
    gate_idx = np.asarray(gate_idx)
    gate_score = np.asarray(gate_score, dtype=np.float32)
    w_htoh4 = np.asarray(w_htoh4, dtype=np.float32)
    w_h4toh = np.asarray(w_h4toh, dtype=np.float32)

    B, d_model = inp.shape
    n_expert, d_ff, _ = w_htoh4.shape
    assert n_expert == NUM_EXPERT
    KO = d_model // P
    FO = d_ff // P

    gi = gate_idx.astype(np.int64)
    order = np.argsort(gi, kind="stable")
    counts = np.bincount(gi, minlength=NUM_EXPERT)
    idx_split = np.split(order, np.cumsum(counts)[:-1])

    chunks = _chunking(counts.max())
    C = sum(chunks)

    # fold per-row gate score into x (scores >= 0 commute with relu)
    scores_flat = gate_score.reshape(-1)
    xs = inp * scores_flat[:, None]

    nc = _build(chunks, KO, FO)

    in_maps = []
    for e in range(NUM_EXPERT):
        idx = idx_split[e]
        cnt = len(idx)
        xT = np.zeros((d_model, C), dtype=np.float32)
        if cnt:
            xT[:, :cnt] = xs[idx].T
        xt_h = np.ascontiguousarray(
            xT.reshape(KO, P, C).transpose(1, 0, 2)).astype(bf16)
        w1_h = np.ascontiguousarray(
            w_htoh4[e].T.reshape(KO, P, d_ff).transpose(1, 0, 2)).astype(bf16)
        # W2^T blocked: w2t[p, do, fo, d] = W2[do*128+d, fo*128+p]
        w2_h = np.ascontiguousarray(
            w_h4toh[e].T.reshape(FO, P, KO, P).transpose(1, 2, 0, 3)
        ).astype(bf16)
        in_maps.append({"xt": xt_h, "w1t": w1_h, "w2t": w2_h})

    from concourse import bass_utils
    res = bass_utils.run_bass_kernel_spmd(nc, in_maps,
                                          core_ids=list(range(N_CORES)))

    _last.update(nc=nc, in_maps=in_maps, res=res, chunks=chunks,
                 KO=KO, FO=FO)

    y_full = np.empty((B, d_model), dtype=np.float32)
    for e in range(NUM_EXPERT):
        idx = idx_split[e]
        if len(idx) == 0:
            continue
        yt_h = np.asarray(res.results[e]["yt"], dtype=np.float32)
        yT = yt_h.transpose(1, 0, 2).reshape(d_model, C)
        y_full[idx] = yT[:, :len(idx)].T

    out = y_full[0::2] + y_full[1::2]
    return np.ascontiguousarray(out, dtype=np.float32)
